# revision 22
# baseline (speedup 1.0000x reference)
"""BiLSTM + prototype-distance kernel for 8 trn2 NeuronCores.

Sharding: 8 cores = 2 directions x 4 core-slots; each core runs TWO
independent time-chunks of the full 32-row batch, INTERLEAVED step by
step so one chunk's serial tail (sigmoid -> c/h algebra -> transpose)
hides under the other chunk's PE matmuls.

The LSTM forgets exponentially (forget gate ~ sigma(z)), so chunks
start from h=c=0 W=32 steps early and warmup outputs are discarded
(validated: h error < 4e-6).  Global chunk j in [0,8) covers window
[60*j, 60*j + 92); real outputs are the last 60 steps (all 92 for j=0).
Core 4*d+q runs chunks 2q (A) and 2q+1 (B) of direction d.

Per-chunk layout (batch 32 -> M=128 full PE util):
  G[32c+b, 128g+j] = gate g (i,f,o,g') of hidden unit 128c+j, batch b.
  hT[j, 32c+b]     = h[b, 128c+j]  (stationary operand of the recurrence).
Recurrence/proto matmuls run 4-way concurrent across PE column groups.
Host combines: out = 2*(xp_f + xp_b) - x2_f - x2_b - ||protos||^2.
"""

import sys
import numpy as np

sys.path.insert(0, "/opt/trn_rl_repo")

import concourse.bass as bass  # noqa: E402
import concourse.tile as tile  # noqa: E402
import concourse.mybir as mybir  # noqa: E402
from concourse import bacc  # noqa: E402
from concourse.bass_utils import run_bass_kernel_spmd  # noqa: E402

F32 = mybir.dt.float32
BF16 = mybir.dt.bfloat16
FP8 = mybir.dt.float8e4
I32 = mybir.dt.int32
SH, SW = 16.0, 8.0          # fp8 pre-scales: h*16, W_hh*8; sigma scale undoes

V, E, HD, P = 50000, 512, 1024, 128
H2 = HD // 2          # 512 per-direction hidden
B, T = 32, 512
WARM = 32             # warmup steps (chunks j>0)
NCH = 8               # time-chunks per direction
L = (T - WARM) // NCH  # 60 real steps per chunk (chunk 0: 92)
NSTEP = L + WARM      # 92 steps per chunk
NG = NSTEP // 4       # 23 granules (4 timesteps each)
GMAP = [0, 1, 3, 2]   # our gate order (i, f, o, g) -> pytorch row-block order
DISABLE = set()


def _arrange_w(w, scale_g):
    """w: (2048, K) -> (4, 128, 2048) tiles: arr[k][kk, 512c+128g+j] =
    w[512*GMAP[g] + 128c + j, 128k + kk] (*2 on the tanh gate)."""
    K = w.shape[1]
    w4 = w.reshape(4, H2, K)[GMAP].copy()      # (gamma, 512, K)
    if scale_g:
        w4[3] *= 2.0
    w5 = w4.reshape(4, 4, 128, K // 128, 128)
    arr = np.transpose(w5, (3, 4, 1, 0, 2)).reshape(K // 128, 128, 2048)
    return np.ascontiguousarray(arr, dtype=np.float32)


def _arrange_b(b_total):
    """-> (128, 2048): bb[p, 512c+128g+j] = b4[g, 128c+j] (broadcast rows)."""
    b4 = b_total.reshape(4, H2)[GMAP].copy()
    b4[3] *= 2.0
    flat = np.transpose(b4.reshape(4, 4, 128), (1, 0, 2)).reshape(2048)
    return np.ascontiguousarray(
        np.broadcast_to(flat[None, :], (128, 2048)), dtype=np.float32)


def _arrange_idx(ids_a, ids_b):
    """two (32, NSTEP) windows -> (128, 2*NG) int32:
    [32*tt + b, ch*NG + g] = ids_ch[b, 4g+tt]."""
    idx = np.zeros((128, 2 * NG), np.int32)
    for ch, ids_w in ((0, ids_a), (1, ids_b)):
        for g in range(NG):
            for tt in range(4):
                idx[32 * tt:32 * tt + 32, ch * NG + g] = ids_w[:, 4 * g + tt]
    return idx


def build_program(n_gran=NG):
    nc = bacc.Bacc("TRN2", target_bir_lowering=False, debug=False)

    emb = nc.dram_tensor("emb", [V, E], F32, kind="ExternalInput").ap()
    idx_d = nc.dram_tensor("idx", [128, 2 * n_gran], I32, kind="ExternalInput").ap()
    wih_d = nc.dram_tensor("wih", [2, 128, 2, 2048], FP8, kind="ExternalInput").ap()
    whh_d = nc.dram_tensor("whh", [4, 128, 2048], BF16, kind="ExternalInput").ap()
    bb_d = nc.dram_tensor("bb", [128, 2048], F32, kind="ExternalInput").ap()
    pt_d = nc.dram_tensor("pt", [128, 512], BF16, kind="ExternalInput").ap()
    idn_d = nc.dram_tensor("idn", [128, 128], BF16, kind="ExternalInput").ap()

    Tloc = 4 * n_gran
    xp_d = nc.dram_tensor("xp", [128, 2 * Tloc * 512], F32,
                          kind="ExternalOutput").ap()
    x2_d = nc.dram_tensor("x2", [128, 2 * Tloc], F32, kind="ExternalOutput").ap()

    with tile.TileContext(nc) as tc:
        _body(tc, n_gran, emb, idx_d, wih_d, whh_d, bb_d, pt_d, idn_d,
              xp_d, x2_d)

    nc.compile()
    return nc


def _body(tc, n_gran, emb, idx_d, wih_d, whh_d, bb_d, pt_d, idn_d, xp_d, x2_d):
    nc = tc.nc
    nstep = 4 * n_gran
    from contextlib import ExitStack
    ctx = ExitStack()
    const = ctx.enter_context(tc.tile_pool(name="const", bufs=1))
    state = ctx.enter_context(tc.tile_pool(name="state", bufs=1))
    work = ctx.enter_context(tc.tile_pool(name="work", bufs=3))
    psum_t = ctx.enter_context(tc.tile_pool(name="pst", bufs=2, space="PSUM"))
    psum_m = ctx.enter_context(tc.tile_pool(name="psm", bufs=2, space="PSUM"))
    psum_g = ctx.enter_context(tc.tile_pool(name="psg", bufs=2, space="PSUM"))
    psum_h = ctx.enter_context(tc.tile_pool(name="psh", bufs=1, space="PSUM"))
    psum_p = ctx.enter_context(tc.tile_pool(name="psp", bufs=1, space="PSUM"))

    # ---- resident tensors -------------------------------------------------
    wih8 = const.tile([128, 2, 2, 2048], FP8)
    whh = const.tile([128, 4 * 2048], BF16)
    bb = const.tile([128, 2048], F32)
    pt = const.tile([128, 4 * 128], BF16)
    identb = const.tile([128, 128], BF16)
    idx = const.tile([128, 2 * n_gran], I32)
    ident = const.tile([128, 128], F32)

    for k in range(4):
        nc.sync.dma_start(whh[:, 2048 * k:2048 * (k + 1)], whh_d[k])
    nc.sync.dma_start(pt[:], pt_d[:])
    for kp in range(2):
        nc.sync.dma_start(wih8[:, kp], wih_d[kp])
    nc.sync.dma_start(bb[:], bb_d[:])
    nc.sync.dma_start(identb[:], idn_d[:])
    nc.sync.dma_start(idx[:], idx_d[:])

    from concourse.masks import make_identity
    make_identity(nc, ident[:])

    # per-chunk state (index by ch in {0,1})
    c_st = [state.tile([128, 128], F32, name=f"c_{c}") for c in (0, 1)]
    hT2 = [state.tile([128, 2 * 128], BF16, name=f"hT_{c}") for c in (0, 1)]
    h_t = [state.tile([128, 128], F32, name=f"h_{c}") for c in (0, 1)]
    emb_ring = [state.tile([128, 4 * 512], F32, name=f"er_{c}") for c in (0, 1)]
    embT8 = [state.tile([128, 2, 2, 2, 128], FP8, name=f"eT_{c}") for c in (0, 1)]
    xg_ring = [state.tile([128, 4 * 2048], BF16, name=f"xg_{c}") for c in (0, 1)]
    out_ring = [state.tile([128, 2 * 4096], F32, name=f"or_{c}") for c in (0, 1)]
    x2buf = state.tile([128, 2 * nstep], F32)
    sq = state.tile([128, 128], F32)

    for ch in (0, 1):
        nc.gpsimd.memset(c_st[ch][:], 0.0)
        nc.gpsimd.memset(hT2[ch][:], 0.0)
        nc.gpsimd.memset(h_t[ch][:], 0.0)
        nc.gpsimd.memset(xg_ring[ch][:], 0.0)
        nc.gpsimd.memset(emb_ring[ch][:], 0.0)
        nc.gpsimd.memset(embT8[ch][:], 0.0)
        nc.gpsimd.memset(out_ring[ch][:], 0.0)
    nc.gpsimd.memset(x2buf[:], 0.0)

    def gather(ch, g):
        s = 512 * (g % 4)
        nc.gpsimd.indirect_dma_start(
            out=emb_ring[ch][:, s:s + 512],
            out_offset=None,
            in_=emb[:],
            in_offset=bass.IndirectOffsetOnAxis(
                ap=idx[:, ch * n_gran + g:ch * n_gran + g + 1], axis=0),
        )

    def phase1_share(ch, g, tt):
        """granule-g xg work for chunk ch, emitted 2 granules early.
        tt 0,1: transpose embeds (2 chunks each); tt 2,3: GEMM (2 windows)."""
        es, ts = 512 * (g % 4), 512 * (g % 2)
        if tt < 2:
            for k in (2 * tt, 2 * tt + 1):
                tp = psum_t.tile([128, 128], F32, padded_shape=[128, 512])
                nc.tensor.matmul(tp[:],
                                 lhsT=emb_ring[ch][:, es + 128 * k:es + 128 * (k + 1)],
                                 rhs=ident[:], is_transpose=True,
                                 start=True, stop=True)
                nc.scalar.mul(embT8[ch][:, g % 2, k // 2, k % 2], tp[:], SH)
        else:
            for c in (2 * (tt - 2), 2 * (tt - 2) + 1):
                mm = psum_m.tile([128, 512], F32)
                for kp in range(2):
                    nc.tensor.matmul(
                        mm[:], lhsT=embT8[ch][:, g % 2, kp],
                        rhs=wih8[:, kp, :, 512 * c:512 * (c + 1)],
                        start=(kp == 0), stop=(kp == 1),
                        perf_mode=mybir.MatmulPerfMode.DoubleRow)
                nc.vector.scalar_tensor_tensor(
                    out=xg_ring[ch][:, 2048 * (g % 4) + 512 * c:
                                    2048 * (g % 4) + 512 * (c + 1)],
                    in0=mm[:], scalar=1.0 / (SH * SW),
                    in1=bb[:, 512 * c:512 * (c + 1)],
                    op0=mybir.AluOpType.mult, op1=mybir.AluOpType.add)

    def inject(ch, t, G):
        tt, gs = t % 4, 2048 * ((t // 4) % 4)
        for c in range(4):
            nc.tensor.matmul(
                G[32 * c:32 * c + 32, :],
                lhsT=identb[:, 32 * tt:32 * tt + 32],
                rhs=xg_ring[ch][:, gs + 512 * c:gs + 512 * (c + 1)],
                start=True, stop=False, tile_position=(0, 32 * c))

    def rec(ch, t, G):
        hs = 128 * ((t - 1) % 2)
        for k in range(4):
            for c in range(4):
                nc.tensor.matmul(
                    G[32 * c:32 * c + 32, :],
                    lhsT=hT2[ch][:, hs + 32 * k:hs + 32 * k + 32],
                    rhs=whh[:, 2048 * k + 512 * c:2048 * k + 512 * (c + 1)],
                    start=False, stop=(k == 3),
                    tile_position=(0, 32 * c))

    def tail_pre(ch, t, G):
        gh = work.tile([128, 512], F32, tag="gh")
        nc.scalar.activation(gh[:], G[:], mybir.ActivationFunctionType.Sigmoid)
        return gh

    def tail_post(ch, t, gh):
        u = work.tile([128, 128], F32, tag="u")
        v = work.tile([128, 128], F32, tag="v")
        nc.vector.scalar_tensor_tensor(
            out=u[:], in0=gh[:, 384:512], scalar=0.5, in1=gh[:, 0:128],
            op0=mybir.AluOpType.subtract, op1=mybir.AluOpType.mult)
        nc.vector.tensor_tensor(out=v[:], in0=gh[:, 128:256], in1=c_st[ch][:],
                                op=mybir.AluOpType.mult)
        nc.vector.scalar_tensor_tensor(
            out=c_st[ch][:], in0=u[:], scalar=2.0, in1=v[:],
            op0=mybir.AluOpType.mult, op1=mybir.AluOpType.add)
        tc_t = work.tile([128, 128], F32, tag="tc")
        nc.scalar.activation(tc_t[:], c_st[ch][:],
                             mybir.ActivationFunctionType.Tanh)
        nc.vector.tensor_tensor(out=h_t[ch][:], in0=gh[:, 256:384], in1=tc_t[:],
                                op=mybir.AluOpType.mult)
        # x2 partial: sq = h*h, accum along free dim -> x2buf[:, ch*nstep+t]
        s = ch * nstep + t
        nc.vector.scalar_tensor_tensor(
            out=sq[:], in0=h_t[ch][:], scalar=1.0, in1=h_t[ch][:],
            op0=mybir.AluOpType.mult, op1=mybir.AluOpType.mult,
            accum_out=x2buf[:, s:s + 1])

    def htrans(ch, t):
        """transpose h -> hT slot t%2 (PE + copy)."""
        hp = psum_h.tile([128, 128], F32, padded_shape=[128, 512])
        nc.tensor.matmul(hp[:], lhsT=h_t[ch][:], rhs=ident[:],
                         is_transpose=True, start=True, stop=True)
        nc.scalar.copy(hT2[ch][:, 128 * (t % 2):128 * (t % 2) + 128], hp[:])

    def proto(ch, t):
        """xp partials: pp[32k+b, 128k'+p] = <h chunk k, proto chunk k'>;
        host uses the k==k' diagonal blocks."""
        hs = 128 * (t % 2)
        pp = psum_p.tile([128, 512], F32)
        nc.tensor.matmul(pp[:], lhsT=hT2[ch][:, hs:hs + 128], rhs=pt[:],
                         start=True, stop=True)
        ring = 4096 * ((t // 8) % 2)
        nc.scalar.copy(
            out_ring[ch][:, ring + 512 * (t % 8):ring + 512 * (t % 8 + 1)],
            pp[:])

    def flush(ch, t):
        """flush proto block [blk0, t) (t multiple of 8, or final partial)."""
        blk0 = (t - 1) // 8 * 8
        n = t - blk0
        ring = 4096 * ((blk0 // 8) % 2)
        base = ch * nstep * 512
        nc.sync.dma_start(xp_d[:, base + blk0 * 512:base + t * 512],
                          out_ring[ch][:, ring:ring + n * 512])

    # ---- main loop --------------------------------------------------------
    for ch in (0, 1):
        for g in range(min(3, n_gran)):
            gather(ch, g)
    for ch in (0, 1):
        for g in range(min(2, n_gran)):
            for tt in range(4):
                phase1_share(ch, g, tt)

    pending = None  # (ch, t) whose htrans is not yet emitted
    for g in range(n_gran):
        for tt in range(4):
            t = 4 * g + tt
            for ch in (0, 1):
                if tt == 0 and g + 3 < n_gran:
                    gather(ch, g + 3)
                G = psum_g.tile([128, 512], F32)
                inject(ch, t, G)
                if g + 2 < n_gran:
                    phase1_share(ch, g + 2, tt)
                rec(ch, t, G)
                if t > 0 and "proto" not in DISABLE:
                    proto(ch, t - 1)
                    if t % 8 == 0:
                        flush(ch, t)
                gh = tail_pre(ch, t, G)
                if pending is not None:
                    htrans(*pending)
                tail_post(ch, t, gh)
                pending = (ch, t)
    htrans(*pending)
    for ch in (0, 1):
        if "proto" not in DISABLE:
            proto(ch, nstep - 1)
            flush(ch, nstep)
    nc.sync.dma_start(x2_d[:], x2buf[:])
    ctx.close()


def _prep_inputs(input_ids, embed_table, w_ih_f, w_hh_f, b_ih_f, b_hh_f,
                 w_ih_b, w_hh_b, b_ih_b, b_hh_b, prototypes, n_gran=NG):
    import ml_dtypes
    bf16 = ml_dtypes.bfloat16
    fp8 = ml_dtypes.float8_e4m3
    ids = np.asarray(input_ids).astype(np.int32)
    emb = np.ascontiguousarray(np.asarray(embed_table, np.float32))
    prot = np.asarray(prototypes, np.float32)
    identb = np.eye(128, dtype=bf16)
    per_dir = {}
    for d, (wi, wh, bi, bh) in enumerate([
            (w_ih_f, w_hh_f, b_ih_f, b_hh_f),
            (w_ih_b, w_hh_b, b_ih_b, b_hh_b)]):
        wih_a = _arrange_w(np.asarray(wi, np.float32), True) * SW
        per_dir[d] = dict(
            wih=np.ascontiguousarray(
                wih_a.reshape(2, 2, 128, 2048).transpose(0, 2, 1, 3)
            ).astype(fp8),
            whh=_arrange_w(np.asarray(wh, np.float32), True).astype(bf16),
            bb=_arrange_b(np.asarray(bi, np.float32)
                          + np.asarray(bh, np.float32)),
            pt=np.ascontiguousarray(
                prot[:, 512 * d:512 * (d + 1)].T.reshape(4, 128, 128)
                .transpose(1, 0, 2).reshape(128, 512)
            ).astype(bf16),
        )
    in_maps = []
    for core in range(8):
        d, q = core // 4, core % 4
        ids_d = ids[:, ::-1] if d == 1 else ids
        wins = []
        for sub in (0, 1):
            j = 2 * q + sub
            wins.append(np.ascontiguousarray(ids_d[:, L * j:L * j + NSTEP]))
        in_maps.append(dict(
            emb=emb,
            idx=_arrange_idx(wins[0], wins[1]),
            wih=per_dir[d]["wih"], whh=per_dir[d]["whh"],
            bb=per_dir[d]["bb"], pt=per_dir[d]["pt"],
            idn=identb,
        ))
    return in_maps


def _combine(results, prototypes, n_gran=NG):
    p2 = (np.asarray(prototypes, np.float32) ** 2).sum(-1)  # (128,)
    out = np.zeros((B, T, 128), np.float32)
    for core in range(8):
        d, q = core // 4, core % 4
        xpf = results[core]["xp"].reshape(4, 32, 2, NSTEP, 4, 128)
        xp4 = np.stack([xpf[k, :, :, :, k, :] for k in range(4)])
        x24 = results[core]["x2"].reshape(4, 32, 2, NSTEP)
        for sub in (0, 1):
            j = 2 * q + sub
            xp = xp4[:, :, sub].sum(0)           # (32, NSTEP, 128)
            x2 = x24[:, :, sub].sum(0)           # (32, NSTEP)
            s0 = 0 if j == 0 else WARM
            t_lo, t_hi = L * j + s0, L * j + NSTEP
            contrib = 2.0 * xp[:, s0:] - x2[:, s0:, None]
            if d == 0:
                out[:, t_lo:t_hi] += contrib
            else:
                out[:, T - t_hi:T - t_lo] += contrib[:, ::-1, :]
    out -= p2[None, None, :]
    return out


_NC_CACHE = {}


def kernel(input_ids, embed_table, w_ih_f, w_hh_f, b_ih_f, b_hh_f,
           w_ih_b, w_hh_b, b_ih_b, b_hh_b, prototypes):
    n_gran = NG
    if n_gran not in _NC_CACHE:
        _NC_CACHE[n_gran] = build_program(n_gran)
    nc = _NC_CACHE[n_gran]
    in_maps = _prep_inputs(input_ids, embed_table, w_ih_f, w_hh_f, b_ih_f,
                           b_hh_f, w_ih_b, w_hh_b, b_ih_b, b_hh_b, prototypes,
                           n_gran)
    res = run_bass_kernel_spmd(nc, in_maps, list(range(8)))
    return _combine(res.results, prototypes, n_gran)


if __name__ == "__main__":
    import time
    t0 = time.time()
    ng = int(sys.argv[1]) if len(sys.argv) > 1 else NG
    nc = build_program(ng)
    print(f"built n_gran={ng} in {time.time()-t0:.1f}s")


# revision 23
# speedup vs baseline: 1.2028x; 1.2028x over previous
"""BiLSTM + prototype-distance kernel for 8 trn2 NeuronCores.

Sharding: 8 cores = 2 directions x 4 core-slots; each core runs TWO
independent time-chunks of the full 32-row batch, INTERLEAVED step by
step so one chunk's serial tail (sigmoid -> c/h algebra -> transpose)
hides under the other chunk's PE matmuls.

The LSTM forgets exponentially (forget gate ~ sigma(z)), so chunks
start from h=c=0 W=32 steps early and warmup outputs are discarded
(validated: h error < 4e-6).  Global chunk j in [0,8) covers window
[60*j, 60*j + 92); real outputs are the last 60 steps (all 92 for j=0).
Core 4*d+q runs chunks 2q (A) and 2q+1 (B) of direction d.

Per-chunk layout (batch 32 -> M=128 full PE util):
  G[32c+b, 128g+j] = gate g (i,f,o,g') of hidden unit 128c+j, batch b.
  hT[j, 32c+b]     = h[b, 128c+j]  (stationary operand of the recurrence).
Recurrence/proto matmuls run 4-way concurrent across PE column groups.
Host combines: out = 2*(xp_f + xp_b) - x2_f - x2_b - ||protos||^2.
"""

import sys
import numpy as np

sys.path.insert(0, "/opt/trn_rl_repo")

import concourse.bass as bass  # noqa: E402
import concourse.tile as tile  # noqa: E402
import concourse.mybir as mybir  # noqa: E402
from concourse import bacc  # noqa: E402
from concourse.bass_utils import run_bass_kernel_spmd  # noqa: E402

F32 = mybir.dt.float32
BF16 = mybir.dt.bfloat16
FP8 = mybir.dt.float8e4
I32 = mybir.dt.int32
SH, SW = 16.0, 8.0          # fp8 pre-scales: h*16, W_hh*8; sigma scale undoes

V, E, HD, P = 50000, 512, 1024, 128
H2 = HD // 2          # 512 per-direction hidden
B, T = 32, 512
WARM = 32             # warmup steps (chunks j>0)
NCH = 8               # time-chunks per direction
L = (T - WARM) // NCH  # 60 real steps per chunk (chunk 0: 92)
NSTEP = L + WARM      # 92 steps per chunk
NG = NSTEP // 4       # 23 granules (4 timesteps each)
GMAP = [0, 1, 3, 2]   # our gate order (i, f, o, g) -> pytorch row-block order
DISABLE = set()


def _arrange_w(w, scale_g):
    """w: (2048, K) -> (4, 128, 2048) tiles: arr[k][kk, 512c+128g+j] =
    w[512*GMAP[g] + 128c + j, 128k + kk] (*2 on the tanh gate)."""
    K = w.shape[1]
    w4 = w.reshape(4, H2, K)[GMAP].copy()      # (gamma, 512, K)
    if scale_g:
        w4[3] *= 2.0
    w5 = w4.reshape(4, 4, 128, K // 128, 128)
    arr = np.transpose(w5, (3, 4, 1, 0, 2)).reshape(K // 128, 128, 2048)
    return np.ascontiguousarray(arr, dtype=np.float32)


def _arrange_b(b_total):
    """-> (128, 2048): bb[p, 512c+128g+j] = b4[g, 128c+j] (broadcast rows)."""
    b4 = b_total.reshape(4, H2)[GMAP].copy()
    b4[3] *= 2.0
    flat = np.transpose(b4.reshape(4, 4, 128), (1, 0, 2)).reshape(2048)
    return np.ascontiguousarray(
        np.broadcast_to(flat[None, :], (128, 2048)), dtype=np.float32)


def _arrange_idx(ids_a, ids_b):
    """two (32, NSTEP) windows -> (128, 2*NG) int32:
    [32*tt + b, ch*NG + g] = ids_ch[b, 4g+tt]."""
    idx = np.zeros((128, 2 * NG), np.int32)
    for ch, ids_w in ((0, ids_a), (1, ids_b)):
        for g in range(NG):
            for tt in range(4):
                idx[32 * tt:32 * tt + 32, ch * NG + g] = ids_w[:, 4 * g + tt]
    return idx


def build_program(n_gran=NG):
    nc = bacc.Bacc("TRN2", target_bir_lowering=False, debug=False)

    emb = nc.dram_tensor("emb", [V, E], F32, kind="ExternalInput").ap()
    idx_d = nc.dram_tensor("idx", [128, 2 * n_gran], I32, kind="ExternalInput").ap()
    wih_d = nc.dram_tensor("wih", [2, 128, 2, 2048], FP8, kind="ExternalInput").ap()
    whh_d = nc.dram_tensor("whh", [4, 128, 2048], BF16, kind="ExternalInput").ap()
    bb_d = nc.dram_tensor("bb", [128, 2048], F32, kind="ExternalInput").ap()
    pt_d = nc.dram_tensor("pt", [4, 128, 128], BF16, kind="ExternalInput").ap()
    idn_d = nc.dram_tensor("idn", [128, 128], BF16, kind="ExternalInput").ap()

    Tloc = 4 * n_gran
    xp_d = nc.dram_tensor("xp", [128, 2 * Tloc * 128], F32,
                          kind="ExternalOutput").ap()
    x2_d = nc.dram_tensor("x2", [128, 2 * Tloc], F32, kind="ExternalOutput").ap()

    with tile.TileContext(nc) as tc:
        _body(tc, n_gran, emb, idx_d, wih_d, whh_d, bb_d, pt_d, idn_d,
              xp_d, x2_d)

    nc.compile()
    return nc


def _body(tc, n_gran, emb, idx_d, wih_d, whh_d, bb_d, pt_d, idn_d, xp_d, x2_d):
    nc = tc.nc
    nstep = 4 * n_gran
    from contextlib import ExitStack
    ctx = ExitStack()
    const = ctx.enter_context(tc.tile_pool(name="const", bufs=1))
    state = ctx.enter_context(tc.tile_pool(name="state", bufs=1))
    work = ctx.enter_context(tc.tile_pool(name="work", bufs=3))
    psum_t = ctx.enter_context(tc.tile_pool(name="pst", bufs=2, space="PSUM"))
    psum_m = ctx.enter_context(tc.tile_pool(name="psm", bufs=2, space="PSUM"))
    psum_g = ctx.enter_context(tc.tile_pool(name="psg", bufs=2, space="PSUM"))
    psum_h = ctx.enter_context(tc.tile_pool(name="psh", bufs=1, space="PSUM"))
    psum_p = ctx.enter_context(tc.tile_pool(name="psp", bufs=1, space="PSUM"))

    # ---- resident tensors -------------------------------------------------
    wih8 = const.tile([128, 2, 2, 2048], FP8)
    whh = const.tile([128, 4 * 2048], BF16)
    bb = const.tile([128, 2048], F32)
    pt = const.tile([128, 4 * 128], BF16)
    identb = const.tile([128, 128], BF16)
    idx = const.tile([128, 2 * n_gran], I32)
    ident = const.tile([128, 128], F32)

    for k in range(4):
        nc.sync.dma_start(whh[:, 2048 * k:2048 * (k + 1)], whh_d[k])
        nc.sync.dma_start(pt[:, 128 * k:128 * (k + 1)], pt_d[k])
    for kp in range(2):
        nc.sync.dma_start(wih8[:, kp], wih_d[kp])
    nc.sync.dma_start(bb[:], bb_d[:])
    nc.sync.dma_start(identb[:], idn_d[:])
    nc.sync.dma_start(idx[:], idx_d[:])

    from concourse.masks import make_identity
    make_identity(nc, ident[:])

    # per-chunk state (index by ch in {0,1})
    c_st = [state.tile([128, 128], F32, name=f"c_{c}") for c in (0, 1)]
    hT2 = [state.tile([128, 2 * 128], BF16, name=f"hT_{c}") for c in (0, 1)]
    h_t = [state.tile([128, 128], F32, name=f"h_{c}") for c in (0, 1)]
    emb_ring = [state.tile([128, 4 * 512], F32, name=f"er_{c}") for c in (0, 1)]
    embT8 = [state.tile([128, 2, 2, 2, 128], FP8, name=f"eT_{c}") for c in (0, 1)]
    xg_ring = [state.tile([128, 4 * 2048], BF16, name=f"xg_{c}") for c in (0, 1)]
    out_ring = [state.tile([128, 2 * 2048], F32, name=f"or_{c}") for c in (0, 1)]
    x2buf = state.tile([128, 2 * nstep], F32)
    sq = state.tile([128, 128], F32)

    for ch in (0, 1):
        nc.gpsimd.memset(c_st[ch][:], 0.0)
        nc.gpsimd.memset(hT2[ch][:], 0.0)
        nc.gpsimd.memset(h_t[ch][:], 0.0)
        nc.gpsimd.memset(xg_ring[ch][:], 0.0)
        nc.gpsimd.memset(emb_ring[ch][:], 0.0)
        nc.gpsimd.memset(embT8[ch][:], 0.0)
        nc.gpsimd.memset(out_ring[ch][:], 0.0)
    nc.gpsimd.memset(x2buf[:], 0.0)

    def gather(ch, g):
        s = 512 * (g % 4)
        nc.gpsimd.indirect_dma_start(
            out=emb_ring[ch][:, s:s + 512],
            out_offset=None,
            in_=emb[:],
            in_offset=bass.IndirectOffsetOnAxis(
                ap=idx[:, ch * n_gran + g:ch * n_gran + g + 1], axis=0),
        )

    def phase1_share(ch, g, tt):
        """granule-g xg work for chunk ch, emitted 2 granules early.
        tt 0,1: transpose embeds (2 chunks each); tt 2,3: GEMM (2 windows)."""
        es, ts = 512 * (g % 4), 512 * (g % 2)
        if tt < 2:
            for k in (2 * tt, 2 * tt + 1):
                tp = psum_t.tile([128, 128], F32, padded_shape=[128, 512])
                nc.tensor.matmul(tp[:],
                                 lhsT=emb_ring[ch][:, es + 128 * k:es + 128 * (k + 1)],
                                 rhs=ident[:], is_transpose=True,
                                 start=True, stop=True)
                nc.scalar.mul(embT8[ch][:, g % 2, k // 2, k % 2], tp[:], SH)
        else:
            for c in (2 * (tt - 2), 2 * (tt - 2) + 1):
                mm = psum_m.tile([128, 512], F32)
                for kp in range(2):
                    nc.tensor.matmul(
                        mm[:], lhsT=embT8[ch][:, g % 2, kp],
                        rhs=wih8[:, kp, :, 512 * c:512 * (c + 1)],
                        start=(kp == 0), stop=(kp == 1),
                        perf_mode=mybir.MatmulPerfMode.DoubleRow)
                nc.vector.scalar_tensor_tensor(
                    out=xg_ring[ch][:, 2048 * (g % 4) + 512 * c:
                                    2048 * (g % 4) + 512 * (c + 1)],
                    in0=mm[:], scalar=1.0 / (SH * SW),
                    in1=bb[:, 512 * c:512 * (c + 1)],
                    op0=mybir.AluOpType.mult, op1=mybir.AluOpType.add)

    def inject(ch, t, G):
        tt, gs = t % 4, 2048 * ((t // 4) % 4)
        for c in range(4):
            nc.tensor.matmul(
                G[32 * c:32 * c + 32, :],
                lhsT=identb[:, 32 * tt:32 * tt + 32],
                rhs=xg_ring[ch][:, gs + 512 * c:gs + 512 * (c + 1)],
                start=True, stop=False, tile_position=(0, 32 * c))

    def rec(ch, t, G):
        hs = 128 * ((t - 1) % 2)
        for k in range(4):
            for c in range(4):
                nc.tensor.matmul(
                    G[32 * c:32 * c + 32, :],
                    lhsT=hT2[ch][:, hs + 32 * k:hs + 32 * k + 32],
                    rhs=whh[:, 2048 * k + 512 * c:2048 * k + 512 * (c + 1)],
                    start=False, stop=(k == 3),
                    tile_position=(0, 32 * c))

    def tail_pre(ch, t, G):
        gh = work.tile([128, 512], F32, tag="gh")
        nc.scalar.activation(gh[:], G[:], mybir.ActivationFunctionType.Sigmoid)
        return gh

    def tail_post(ch, t, gh):
        u = work.tile([128, 128], F32, tag="u")
        v = work.tile([128, 128], F32, tag="v")
        nc.vector.scalar_tensor_tensor(
            out=u[:], in0=gh[:, 384:512], scalar=0.5, in1=gh[:, 0:128],
            op0=mybir.AluOpType.subtract, op1=mybir.AluOpType.mult)
        nc.vector.tensor_tensor(out=v[:], in0=gh[:, 128:256], in1=c_st[ch][:],
                                op=mybir.AluOpType.mult)
        nc.vector.scalar_tensor_tensor(
            out=c_st[ch][:], in0=u[:], scalar=2.0, in1=v[:],
            op0=mybir.AluOpType.mult, op1=mybir.AluOpType.add)
        tc_t = work.tile([128, 128], F32, tag="tc")
        nc.scalar.activation(tc_t[:], c_st[ch][:],
                             mybir.ActivationFunctionType.Tanh)
        nc.vector.tensor_tensor(out=h_t[ch][:], in0=gh[:, 256:384], in1=tc_t[:],
                                op=mybir.AluOpType.mult)
        # x2 partial: sq = h*h, accum along free dim -> x2buf[:, ch*nstep+t]
        s = ch * nstep + t
        nc.vector.scalar_tensor_tensor(
            out=sq[:], in0=h_t[ch][:], scalar=1.0, in1=h_t[ch][:],
            op0=mybir.AluOpType.mult, op1=mybir.AluOpType.mult,
            accum_out=x2buf[:, s:s + 1])

    def htrans(ch, t):
        """transpose h -> hT slot t%2 (PE + copy)."""
        hp = psum_h.tile([128, 128], F32, padded_shape=[128, 512])
        nc.tensor.matmul(hp[:], lhsT=h_t[ch][:], rhs=ident[:],
                         is_transpose=True, start=True, stop=True)
        nc.scalar.copy(hT2[ch][:, 128 * (t % 2):128 * (t % 2) + 128], hp[:])

    def proto(ch, t):
        """xp partials: pp[32k+b, p] = <h chunk k, proto>."""
        hs = 128 * (t % 2)
        pp = psum_p.tile([128, 128], F32, padded_shape=[128, 512])
        for k in range(4):
            nc.tensor.matmul(
                pp[32 * k:32 * k + 32, :],
                lhsT=hT2[ch][:, hs + 32 * k:hs + 32 * k + 32],
                rhs=pt[:, 128 * k:128 * (k + 1)],
                start=True, stop=True, tile_position=(0, 32 * k))
        ring = 2048 * ((t // 16) % 2)
        nc.vector.tensor_scalar_mul(
            out_ring[ch][:, ring + 128 * (t % 16):ring + 128 * (t % 16 + 1)],
            pp[:], 1.0)

    def flush(ch, t):
        """flush proto block [blk0, t) (t multiple of 16, or final partial)."""
        blk0 = (t - 1) // 16 * 16
        n = t - blk0
        ring = 2048 * ((blk0 // 16) % 2)
        base = ch * nstep * 128
        nc.sync.dma_start(xp_d[:, base + blk0 * 128:base + t * 128],
                          out_ring[ch][:, ring:ring + n * 128])

    # ---- main loop --------------------------------------------------------
    for ch in (0, 1):
        for g in range(min(3, n_gran)):
            gather(ch, g)
    for ch in (0, 1):
        for g in range(min(2, n_gran)):
            for tt in range(4):
                phase1_share(ch, g, tt)

    pending = None  # (ch, t) whose htrans is not yet emitted
    for g in range(n_gran):
        for tt in range(4):
            t = 4 * g + tt
            for ch in (0, 1):
                if tt == 0 and g + 3 < n_gran:
                    gather(ch, g + 3)
                G = psum_g.tile([128, 512], F32)
                inject(ch, t, G)
                if g + 2 < n_gran:
                    phase1_share(ch, g + 2, tt)
                rec(ch, t, G)
                if t > 0 and "proto" not in DISABLE:
                    proto(ch, t - 1)
                    if t % 16 == 0:
                        flush(ch, t)
                gh = tail_pre(ch, t, G)
                if pending is not None:
                    htrans(*pending)
                tail_post(ch, t, gh)
                pending = (ch, t)
    htrans(*pending)
    for ch in (0, 1):
        if "proto" not in DISABLE:
            proto(ch, nstep - 1)
            flush(ch, nstep)
    nc.sync.dma_start(x2_d[:], x2buf[:])
    ctx.close()


def _prep_inputs(input_ids, embed_table, w_ih_f, w_hh_f, b_ih_f, b_hh_f,
                 w_ih_b, w_hh_b, b_ih_b, b_hh_b, prototypes, n_gran=NG):
    import ml_dtypes
    bf16 = ml_dtypes.bfloat16
    fp8 = ml_dtypes.float8_e4m3
    ids = np.asarray(input_ids).astype(np.int32)
    emb = np.ascontiguousarray(np.asarray(embed_table, np.float32))
    prot = np.asarray(prototypes, np.float32)
    identb = np.eye(128, dtype=bf16)
    per_dir = {}
    for d, (wi, wh, bi, bh) in enumerate([
            (w_ih_f, w_hh_f, b_ih_f, b_hh_f),
            (w_ih_b, w_hh_b, b_ih_b, b_hh_b)]):
        wih_a = _arrange_w(np.asarray(wi, np.float32), True) * SW
        per_dir[d] = dict(
            wih=np.ascontiguousarray(
                wih_a.reshape(2, 2, 128, 2048).transpose(0, 2, 1, 3)
            ).astype(fp8),
            whh=_arrange_w(np.asarray(wh, np.float32), True).astype(bf16),
            bb=_arrange_b(np.asarray(bi, np.float32)
                          + np.asarray(bh, np.float32)),
            pt=np.ascontiguousarray(
                prot[:, 512 * d:512 * (d + 1)].T.reshape(4, 128, 128)
            ).astype(bf16),
        )
    in_maps = []
    for core in range(8):
        d, q = core // 4, core % 4
        ids_d = ids[:, ::-1] if d == 1 else ids
        wins = []
        for sub in (0, 1):
            j = 2 * q + sub
            wins.append(np.ascontiguousarray(ids_d[:, L * j:L * j + NSTEP]))
        in_maps.append(dict(
            emb=emb,
            idx=_arrange_idx(wins[0], wins[1]),
            wih=per_dir[d]["wih"], whh=per_dir[d]["whh"],
            bb=per_dir[d]["bb"], pt=per_dir[d]["pt"],
            idn=identb,
        ))
    return in_maps


def _combine(results, prototypes, n_gran=NG):
    p2 = (np.asarray(prototypes, np.float32) ** 2).sum(-1)  # (128,)
    out = np.zeros((B, T, 128), np.float32)
    for core in range(8):
        d, q = core // 4, core % 4
        xp4 = results[core]["xp"].reshape(4, 32, 2, NSTEP, 128)
        x24 = results[core]["x2"].reshape(4, 32, 2, NSTEP)
        for sub in (0, 1):
            j = 2 * q + sub
            xp = xp4[:, :, sub].sum(0)           # (32, NSTEP, 128)
            x2 = x24[:, :, sub].sum(0)           # (32, NSTEP)
            s0 = 0 if j == 0 else WARM
            t_lo, t_hi = L * j + s0, L * j + NSTEP
            contrib = 2.0 * xp[:, s0:] - x2[:, s0:, None]
            if d == 0:
                out[:, t_lo:t_hi] += contrib
            else:
                out[:, T - t_hi:T - t_lo] += contrib[:, ::-1, :]
    out -= p2[None, None, :]
    return out


_NC_CACHE = {}


def kernel(input_ids, embed_table, w_ih_f, w_hh_f, b_ih_f, b_hh_f,
           w_ih_b, w_hh_b, b_ih_b, b_hh_b, prototypes):
    n_gran = NG
    if n_gran not in _NC_CACHE:
        _NC_CACHE[n_gran] = build_program(n_gran)
    nc = _NC_CACHE[n_gran]
    in_maps = _prep_inputs(input_ids, embed_table, w_ih_f, w_hh_f, b_ih_f,
                           b_hh_f, w_ih_b, w_hh_b, b_ih_b, b_hh_b, prototypes,
                           n_gran)
    res = run_bass_kernel_spmd(nc, in_maps, list(range(8)))
    return _combine(res.results, prototypes, n_gran)


if __name__ == "__main__":
    import time
    t0 = time.time()
    ng = int(sys.argv[1]) if len(sys.argv) > 1 else NG
    nc = build_program(ng)
    print(f"built n_gran={ng} in {time.time()-t0:.1f}s")
